# revision 7
# baseline (speedup 1.0000x reference)
"""Distortion loss (mip-NeRF 360 style) on 8 Trainium2 NeuronCores.

Math: for each ray with sorted interval boundaries t (N+1 values given as
intervals (t_i, t_{i+1})), s = (t - t_near) / (t_far - t_near),
  u_i   = (s_i + s_{i+1}) / 2           (midpoints, SORTED because t sorted)
  loss  = sum_ij w_i w_j |u_i - u_j| + (1/3) sum_i w_i^2 (s_{i+1} - s_i)

Because u is sorted along N, the O(N^2) pairwise term collapses to O(N):
  sum_ij w_i w_j |u_i - u_j| = 2 sum_i w_i (u_i cumW_i - cumWU_i)
with inclusive prefix sums cumW = cumsum(w), cumWU = cumsum(w*u).
Using m = 2u = s0 + s1, the factors of 2 cancel:
  inter = sum_i (w m)_i cumW_i - sum_i w_i cumsum(w*m)_i
And s-space affine rescaling factors out entirely:
  loss = inv * (inter_t + intra_t / 3),  inv = 1 / (t_far - t_near)
so everything is computed in t-space with one final per-ray scale.

Sharding: embarrassingly data-parallel over rays; B=4096 rays split into 8
shards of 512; each core processes 4 groups of 128 rays (128 partitions).
"""

import numpy as np

B, N = 4096, 128
NCORES = 8
BS = B // NCORES  # 512 rays per core
P = 128  # partitions
G = BS // P  # 4 ray-groups per core

# "scan": DVE tensor_tensor_scan prefix sums.
# "matmul": PE sign-matrix matmul for the pairwise term.
VARIANT = "matmul"

_CACHE = {}


def _build(variant):
    from contextlib import ExitStack

    import concourse.bacc as bacc
    import concourse.mybir as mybir
    import concourse.tile as tile
    from concourse.masks import make_identity

    ALU = mybir.AluOpType
    FP32 = mybir.dt.float32

    nc = bacc.Bacc("TRN2", target_bir_lowering=False, debug=False)

    t_d = nc.dram_tensor("t_inters", [BS, N, 2], FP32, kind="ExternalInput")
    w_d = nc.dram_tensor("weights", [BS, N], FP32, kind="ExternalInput")
    tn_d = nc.dram_tensor("t_near", [BS, 1], FP32, kind="ExternalInput")
    tf_d = nc.dram_tensor("t_far", [BS, 1], FP32, kind="ExternalInput")
    o_d = nc.dram_tensor("out", [BS], FP32, kind="ExternalOutput")

    with tile.TileContext(nc) as tc, ExitStack() as ctx:
        pool = ctx.enter_context(tc.tile_pool(name="main", bufs=1))
        scr_pool = ctx.enter_context(tc.tile_pool(name="scr", bufs=4))
        psum = ctx.enter_context(tc.tile_pool(name="psum", bufs=3, space="PSUM"))
        psum1 = ctx.enter_context(tc.tile_pool(name="psum1", bufs=1, space="PSUM"))

        # ---- loads (ray index = g*128 + p) ----
        t4 = pool.tile([P, G, N, 2], FP32)
        nc.sync.dma_start(t4[:], t_d.ap().rearrange("(g p) n k -> p g n k", p=P))
        w4 = pool.tile([P, G, N], FP32)
        nc.sync.dma_start(w4[:], w_d.ap().rearrange("(g p) n -> p g n", p=P))
        tnT = pool.tile([G, P], FP32)
        nc.scalar.dma_start(tnT[:], tn_d.ap().rearrange("(g p) one -> g (p one)", g=G))
        tfT = pool.tile([G, P], FP32)
        nc.scalar.dma_start(tfT[:], tf_d.ap().rearrange("(g p) one -> g (p one)", g=G))

        # ---- per-ray scale factor inv = 1/(t_far - t_near), in [G, P] layout ----
        dT = scr_pool.tile([G, P], FP32)
        nc.vector.tensor_tensor(dT[:], tfT[:], tnT[:], ALU.subtract)
        invT = pool.tile([G, P], FP32)
        nc.vector.reciprocal(invT[:], dT[:])

        s0 = t4[:, :, :, 0]  # [P, G, N] interval starts (t-space)
        s1 = t4[:, :, :, 1]  # [P, G, N] interval ends

        # ---- elementwise (GPSIMD + ACT off the DVE critical path) ----
        m4 = pool.tile([P, G, N], FP32)
        nc.gpsimd.tensor_tensor(m4[:], s0, s1, ALU.add)  # m = s0+s1 = 2u
        du4 = pool.tile([P, G, N], FP32)
        nc.gpsimd.tensor_tensor(du4[:], s1, s0, ALU.subtract)
        w24 = pool.tile([P, G, N], FP32)
        nc.scalar.square(w24[:], w4[:])
        wm4 = pool.tile([P, G, N], FP32)
        nc.vector.tensor_tensor(wm4[:], w4[:], m4[:], ALU.mult)

        identity = pool.tile([P, P], FP32)
        make_identity(nc, identity[:])

        loss_cols = pool.tile([P, G], FP32)
        accA = pool.tile([P, G], FP32)
        accB = pool.tile([P, G], FP32)
        accC = pool.tile([P, G], FP32)

        if variant == "scan":
            zeros = pool.tile([P, N], FP32)
            nc.vector.memset(zeros[:], 0.0)
            cumW = pool.tile([P, G, N], FP32)
            cumWM = pool.tile([P, G, N], FP32)
            for g in range(G):
                # inclusive prefix sums along N
                nc.vector.tensor_tensor_scan(
                    cumW[:, g], w4[:, g], zeros[:], 0.0, ALU.add, ALU.add
                )
                nc.vector.tensor_tensor_scan(
                    cumWM[:, g], wm4[:, g], zeros[:], 0.0, ALU.add, ALU.add
                )
                scr = scr_pool.tile([P, N], FP32, tag="scr")
                nc.vector.scalar_tensor_tensor(
                    out=scr[:], in0=wm4[:, g], scalar=1.0, in1=cumW[:, g],
                    op0=ALU.mult, op1=ALU.mult,
                    accum_out=accA[:, g : g + 1],
                )
                scr2 = scr_pool.tile([P, N], FP32, tag="scr")
                nc.vector.scalar_tensor_tensor(
                    out=scr2[:], in0=w4[:, g], scalar=1.0, in1=cumWM[:, g],
                    op0=ALU.mult, op1=ALU.mult,
                    accum_out=accB[:, g : g + 1],
                )
                scr3 = scr_pool.tile([P, N], FP32, tag="scr")
                nc.vector.scalar_tensor_tensor(
                    out=scr3[:], in0=w24[:, g], scalar=1.0 / 3.0, in1=du4[:, g],
                    op0=ALU.mult, op1=ALU.mult,
                    accum_out=accC[:, g : g + 1],
                )
        else:  # "matmul" variant: pairwise term via PE sign matrix
            # sg[j, i] = sign(i - j); then sum_ij w_i m_i sign(i-j) w_j
            #   = sum_i (w m)_i (S w)_i   with (S w)_i = sum_j sign(i-j) w_j
            sg = pool.tile([P, P], FP32)
            nc.gpsimd.memset(sg[:], -1.0)
            # keep -1.0 where (p - f) >= 0 i.e. f <= p; else fill +1
            nc.gpsimd.affine_select(
                out=sg[:], in_=sg[:], compare_op=ALU.is_ge, fill=1.0,
                base=0, pattern=[[-1, P]], channel_multiplier=1,
            )
            # zero the diagonal: keep where (p - f) != 0
            nc.gpsimd.affine_select(
                out=sg[:], in_=sg[:], compare_op=ALU.not_equal, fill=0.0,
                base=0, pattern=[[-1, P]], channel_multiplier=1,
            )
            for g in range(G):
                wT_ps = psum.tile([P, P], FP32, tag="wT")
                nc.tensor.transpose(wT_ps[:], w4[:, g], identity[:])
                wT = scr_pool.tile([P, P], FP32, tag="wT_sb")
                nc.scalar.copy(wT[:], wT_ps[:])
                sw_ps = psum.tile([P, P], FP32, tag="sw")
                nc.tensor.matmul(sw_ps[:], wT[:], sg[:])  # [b, i] = (S w)_i
                scr = scr_pool.tile([P, N], FP32, tag="scr")
                nc.vector.scalar_tensor_tensor(
                    out=scr[:], in0=wm4[:, g], scalar=1.0, in1=sw_ps[:],
                    op0=ALU.mult, op1=ALU.mult,
                    accum_out=accA[:, g : g + 1],
                )
                scr3 = scr_pool.tile([P, N], FP32, tag="scr")
                nc.vector.scalar_tensor_tensor(
                    out=scr3[:], in0=w24[:, g], scalar=1.0 / 3.0, in1=du4[:, g],
                    op0=ALU.mult, op1=ALU.mult,
                    accum_out=accC[:, g : g + 1],
                )

        # ---- combine partial sums -> per-ray loss (t-space) ----
        if variant == "scan":
            # loss = A - B + C
            nc.vector.scalar_tensor_tensor(
                out=loss_cols[:], in0=accB[:], scalar=-1.0, in1=accA[:],
                op0=ALU.mult, op1=ALU.add,
            )
            nc.vector.tensor_tensor(loss_cols[:], loss_cols[:], accC[:], ALU.add)
        else:
            # loss = A + C
            nc.vector.tensor_tensor(loss_cols[:], accA[:], accC[:], ALU.add)

        # ---- transpose [P, G] -> [G, P], scale by inv, store ----
        lossT_ps = psum1.tile([G, P], FP32, tag="lossT")
        nc.tensor.transpose(lossT_ps[:], loss_cols[:], identity[:])
        outT = pool.tile([G, P], FP32)
        nc.vector.tensor_tensor(outT[:], lossT_ps[:], invT[:], ALU.mult)
        nc.sync.dma_start(o_d.ap().rearrange("(g p) -> g p", g=G), outT[:])

    nc.compile()
    return nc


def _get_nc(variant=None):
    variant = variant or VARIANT
    if variant not in _CACHE:
        _CACHE[variant] = _build(variant)
    return _CACHE[variant]


def kernel(t_inters, weights, t_near, t_far, _variant=None, _trace=False):
    from concourse.bass_utils import run_bass_kernel_spmd

    nc = _get_nc(_variant)

    t_inters = np.ascontiguousarray(np.asarray(t_inters, dtype=np.float32))
    weights = np.ascontiguousarray(np.asarray(weights, dtype=np.float32))
    t_near = np.ascontiguousarray(np.asarray(t_near, dtype=np.float32))
    t_far = np.ascontiguousarray(np.asarray(t_far, dtype=np.float32))

    in_maps = []
    for c in range(NCORES):
        s = slice(c * BS, (c + 1) * BS)
        in_maps.append(
            {
                "t_inters": t_inters[s],
                "weights": weights[s],
                "t_near": t_near[s],
                "t_far": t_far[s],
            }
        )

    res = run_bass_kernel_spmd(nc, in_maps, core_ids=list(range(NCORES)), trace=_trace)
    out = np.concatenate([res.results[c]["out"] for c in range(NCORES)])
    if _trace:
        return out, res
    return out
